# revision 21
# baseline (speedup 1.0000x reference)
"""Trainium2 Bass kernel for nn_CapGATattentionGRU (8-core SPMD).

Math notes exploited here:
- The reference GRU scans a length-1 sequence with h0 = 0, so the
  (3F x F) W_hh matmuls reduce to their biases b_hh.  Only W_ih0/W_ih1
  (100 MB total) need to be streamed.
- Tensor-parallel sharding: each core owns 256 output features per gate
  (columns of gi) for both GRU layers; hidden states are AllGathered
  between layers.  Everything after the GRU (attention over T=12, GAT on
  128 nodes, capsule routing) is tiny and runs replicated on all cores.
- All matmul inputs are bf16 (fp32 PSUM accumulation); fp32 matmuls on
  TRN2 run in LOW_HIGH dual-pass mode (~2x columns + no drain overlap),
  measured ~5-10x slower for these shapes.
"""

import os
import numpy as np

I, H, T, F = 128, 16, 12, 2048
NCORES = 8
GPC = F // NCORES          # 256 gate-features per core
KT = F // 128              # 16 k-tiles of the contraction dim
NCHUNK = 8                 # weight DMA chunks per layer (split over 2 HWDGE rings)

_STATE = {}


# --------------------------------------------------------------------------
# device module
# --------------------------------------------------------------------------
def _build_module():
    from contextlib import ExitStack

    import concourse.bass as bass
    import concourse.tile as tile
    from concourse import bacc, mybir
    from concourse.masks import make_identity

    dt = mybir.dt.float32
    db = mybir.dt.bfloat16
    X = mybir.AxisListType.X
    AF = mybir.ActivationFunctionType
    OP = mybir.AluOpType
    AP = bass.AP

    nc = bacc.Bacc(
        "TRN2",
        target_bir_lowering=False,
        debug=False,
        num_devices=NCORES,
    )

    def din(name, shape, dd=dt):
        return nc.dram_tensor(name, list(shape), dd, kind="ExternalInput").ap()

    w_dram = [
        [din(f"w{layer}_{j}", (128, KT * 3 * GPC // NCHUNK), db) for j in range(NCHUNK)]
        for layer in range(2)
    ]  # each (128, 3072) bf16: k-major [k, j] with j in 0..768
    xT_d = din("xT", (128, KT * T), db)
    bias_d = [din(f"bias{layer}", (128, 8)) for layer in range(2)]
    wattT_d = din("wattT", (T, T), db)
    battbc_d = din("battbc", (128, T))
    gat_d = []
    for layer in range(2):
        gat_d.append(
            dict(
                wlT=din(f"wlT{layer}", (H, H), db),
                wrT=din(f"wrT{layer}", (H, H), db),
                bl=din(f"bl{layer}", (H, 1)),
                br=din(f"br{layer}", (H, 1)),
                gb=din(f"gb{layer}", (H, 1)),
                abc=din(f"abc{layer}", (128, H)),
            )
        )
    wc1T_d = din("wc1T", (H, 2048), db)
    wc2T_d = din("wc2T", (H, 2048), db)
    wfusT_d = din("wfusT", (H, 128), db)
    bfus_d = din("bfus", (128, 128))
    out_d = nc.dram_tensor("out", [128, 128], dt, kind="ExternalOutput").ap()

    with ExitStack() as ctx:
        tc = ctx.enter_context(tile.TileContext(nc))
        const = ctx.enter_context(tc.tile_pool(name="const", bufs=1))
        work = ctx.enter_context(tc.tile_pool(name="work", bufs=1))
        psum = ctx.enter_context(tc.tile_pool(name="psum", bufs=1, space="PSUM"))
        dram = ctx.enter_context(tc.tile_pool(name="dram", bufs=1, space="DRAM"))

        def wtile(shape, tag, dd=dt):
            return work.tile(list(shape), dd, tag=tag, name=tag)

        def ptile(shape, i, dd=dt):
            return psum.tile(list(shape), dd, tag=f"P{i}", name=f"P{i}")

        def ttile(shape, dd=dt):
            return psum.tile(list(shape), dd, tag="tr", name="tr", bufs=2)

        def bc_free(ap, dims):
            """Manual AP: keep partition dim, replace free dims with [step,count]s."""
            return AP(tensor=ap.tensor, offset=ap.offset, ap=[ap.ap[0]] + list(dims))

        # ---- big weight DMAs first (HWDGE, streams in order) -------------
        w_sb = []
        for layer in range(2):
            tiles = []
            for j in range(NCHUNK):
                t = const.tile([128, KT * 3 * GPC // NCHUNK], db,
                               tag=f"w{layer}_{j}", name=f"w{layer}_{j}")
                eng = nc.sync if j % 2 == 0 else nc.scalar
                eng.dma_start(out=t[:], in_=w_dram[layer][j])
                tiles.append(t)
            w_sb.append(tiles)

        # ---- small constants (SWDGE queues, overlap the weight stream) ---
        def load(ap_dram, tag):
            t = const.tile(list(ap_dram.shape), ap_dram.dtype, tag=tag, name=tag)
            nc.gpsimd.dma_start(out=t[:], in_=ap_dram)
            return t

        xT_sb = load(xT_d, "xT")
        bias_sb = [load(bias_d[0], "bias0"), load(bias_d[1], "bias1")]
        wattT_sb = load(wattT_d, "wattT")
        battbc_sb = load(battbc_d, "battbc")
        gat_sb = []
        for layer in range(2):
            gat_sb.append({k: load(v, f"gat{layer}_{k}") for k, v in gat_d[layer].items()})
        wc1T_sb = load(wc1T_d, "wc1T")
        wc2T_sb = load(wc2T_d, "wc2T")
        wfusT_sb = load(wfusT_d, "wfusT")
        bfus_sb = load(bfus_d, "bfus")

        ident = const.tile([128, 128], dt, tag="ident", name="ident")
        make_identity(nc, ident[:])
        identb = const.tile([128, 128], db, tag="identb", name="identb")
        make_identity(nc, identb[:])
        ones1 = const.tile([1, 128], db, tag="ones1", name="ones1")
        nc.vector.memset(ones1[:], 1.0)
        eps_t = const.tile([128, 1], dt, tag="eps_t", name="eps_t")
        nc.vector.memset(eps_t[:], 1e-8)
        one_c = const.tile([128, 1], db, tag="one_c", name="one_c")
        nc.vector.memset(one_c[:], 1.0)

        # ---- GRU layers --------------------------------------------------
        h1T_sb = wtile((128, KT * T), "h1T", db)
        embT_bf = wtile((128, KT * T), "embT", db)
        # layer 0 gathers the (f, t) layout only; layer 1 additionally
        # gathers pre-transposed (t, f) blocks so emb lands in both layouts
        # with no post-gather transposes.
        d_slice = [
            dram.tile([2 * 128, T], db, tag="dsl0", name="dsl0"),
            dram.tile([2 * 128 + 2 * 128, T], db, tag="dsl1", name="dsl1"),
        ]
        d_full = [
            dram.tile([F, T], db, tag="dfull0", name="dfull0"),
            dram.tile([2 * F, T], db, tag="dfull1", name="dfull1"),
        ]

        for layer in range(2):
            rhs3 = (xT_sb if layer == 0 else h1T_sb)[:].rearrange(
                "p (k t) -> p k t", k=KT
            )
            ps = [ptile([128, T], g * 2 + b) for g in range(3) for b in range(2)]
            for k in range(KT):
                ch, kk = k // 2, k % 2
                wv = w_sb[layer][ch][:].rearrange("p (k2 j) -> p k2 j", k2=2)
                for g in range(3):
                    for b in range(2):
                        nc.tensor.matmul(
                            ps[g * 2 + b][:],
                            lhsT=wv[:, kk, g * GPC + b * 128 : g * GPC + (b + 1) * 128],
                            rhs=rhs3[:, k, :],
                            start=(k == 0),
                            stop=(k == KT - 1),
                        )
            bl_sb = bias_sb[layer]
            r_t, zc_t, t_t, n_t, h_t = {}, {}, {}, {}, {}
            for b in range(2):  # batch sigmoids (one ACT table load)
                r_t[b] = wtile((128, T), f"r{b}")
                nc.scalar.activation(r_t[b][:], ps[0 * 2 + b][:], AF.Sigmoid,
                                     bias=bl_sb[:, 0 + b : 1 + b])
                zc_t[b] = wtile((128, T), f"zc{b}")
                nc.scalar.activation(zc_t[b][:], ps[1 * 2 + b][:], AF.Sigmoid,
                                     bias=bl_sb[:, 2 + b : 3 + b], scale=-1.0)
            for b in range(2):
                t_t[b] = wtile((128, T), f"t{b}")
                nc.vector.scalar_tensor_tensor(
                    out=t_t[b][:], in0=r_t[b][:], scalar=bl_sb[:, 6 + b : 7 + b],
                    in1=ps[2 * 2 + b][:], op0=OP.mult, op1=OP.add,
                )
            for b in range(2):
                n_t[b] = wtile((128, T), f"n{b}")
                nc.scalar.activation(n_t[b][:], t_t[b][:], AF.Tanh,
                                     bias=bl_sb[:, 4 + b : 5 + b])
            for b in range(2):
                h_t[b] = wtile((128, T), f"h{b}", db)
                if layer == 0:
                    nc.vector.tensor_mul(h_t[b][:], zc_t[b][:], n_t[b][:])
                else:
                    hf_t = wtile((128, T), f"hf{b}")
                    nc.vector.tensor_mul(hf_t[:], zc_t[b][:], n_t[b][:])
                    nc.scalar.activation(h_t[b][:], hf_t[:], AF.Relu)  # emb = relu
                nc.gpsimd.dma_start(
                    out=d_slice[layer][b * 128 : (b + 1) * 128, :], in_=h_t[b][:]
                )
                if layer == 1:
                    # also ship the (t, f) layout: transpose the relu'd block
                    trh = ttile([T, 128], db)
                    nc.tensor.transpose(trh[:], h_t[b][:], identb[:])
                    hn_t = wtile((T, 128), f"hn{b}", db)
                    nc.scalar.copy(hn_t[:], trh[:])
                    nat = d_slice[1][256 + b * 128 : 256 + (b + 1) * 128, :]
                    nat = nat.rearrange("a b -> (a b)").rearrange(
                        "(t j) -> t j", t=T
                    )  # (12, 128) region, t-major
                    nc.gpsimd.dma_start(out=nat, in_=hn_t[:])
            nc.gpsimd.collective_compute(
                "AllGather",
                OP.bypass,
                replica_groups=[list(range(NCORES))],
                ins=[d_slice[layer][:].opt()],
                outs=[d_full[layer][:].opt()],
            )
            if layer == 0:
                h1T3 = h1T_sb[:].rearrange("p (k t) -> p k t", k=KT)
                dfull3 = d_full[0][:].rearrange("(k p) t -> p k t", p=128)
                nc.gpsimd.dma_start(out=h1T3[:, 0:8, :], in_=dfull3[:, 0:8, :])
                nc.gpsimd.dma_start(out=h1T3[:, 8:16, :], in_=dfull3[:, 8:16, :])

        # d_full[1] is (8 cores) x [2*128 rows of (f,t) slice | 2*128 rows of
        # flat (t,256) nat slice].  Element layouts (flat offsets, bf16):
        #   fT half:  core c, j in [0,256), t:   c*6144 + j*12 + t
        #   nat half: core c, t, j in [0,256):   c*6144 + 3072 + t*256 + j
        full1 = d_full[1][:].rearrange("a b -> (a b)")
        emb_nat = wtile((T, F), "emb_nat", db)
        embnat3 = emb_nat[:].rearrange("t (c j) -> t c j", c=8)
        for cg in range(2):  # aw matmuls consume emb_nat: land it first, halved
            embnat_in = AP(tensor=full1.tensor,
                           offset=full1.offset + 3072 + cg * 4 * 6144,
                           ap=[[256, 12], [6144, 4], [1, 256]])
            nc.gpsimd.dma_start(out=embnat3[:, cg * 4 : (cg + 1) * 4, :],
                                in_=embnat_in)
        embT3v = embT_bf[:].rearrange("p (c u t) -> p c u t", c=8, u=2)
        for u in range(2):
            embT_in = AP(tensor=full1.tensor, offset=full1.offset + u * 1536,
                         ap=[[12, 128], [6144, 8], [1, 12]])
            nc.gpsimd.dma_start(out=embT3v[:, :, u, :], in_=embT_in)

        # ---- attention over T --------------------------------------------
        # aw[f, t'] = sum_t emb[t, f] W_att[t', t] computed directly in
        # (f-part, t'-free) orientation: lhsT = emb_nat slice, rhs = W_att^T.
        aw_sb = wtile((128, KT * T), "aw")
        aw3 = aw_sb[:].rearrange("p (k t) -> p k t", k=KT)
        for k in range(KT):
            pa = ttile([128, T])
            nc.tensor.matmul(pa[:], lhsT=emb_nat[:, k * 128 : (k + 1) * 128],
                             rhs=wattT_sb[:], start=True, stop=True)
            nc.scalar.copy(aw3[:, k, :], pa[:])
        battbc_ap = battbc_sb[:]
        nc.vector.tensor_tensor(
            aw3, aw3, bc_free(battbc_ap, [[0, KT], [1, T]]), OP.add
        )

        # |aw| <= ~1 so exp cannot overflow: skip the max-subtraction pass.
        exs = wtile((128, KT * T), "exs", db)
        ex3 = exs[:].rearrange("p (k t) -> p k t", k=KT)
        nc.scalar.activation(exs[:], aw_sb[:], AF.Exp)
        sm = wtile((128, KT), "sm")
        nc.vector.reduce_sum(out=sm[:], in_=ex3, axis=X)
        rs = wtile((128, KT), "rs")
        nc.vector.reciprocal(rs[:], sm[:])
        pe = wtile((128, KT * T), "pe")
        nc.vector.tensor_mul(pe[:], exs[:], embT_bf[:])
        num = wtile((128, KT), "num")
        nc.vector.reduce_sum(out=num[:], in_=pe[:].rearrange("p (k t) -> p k t", k=KT),
                             axis=X)
        attn = wtile((128, KT), "attn")
        nc.vector.tensor_mul(attn[:], num[:], rs[:])
        # Layer-1 features are h-major permuted (f' = h*128 + i), so the
        # (p, k) attention layout IS att_vec (i, h) directly.
        att_vec = wtile((128, H), "att_vec", db)
        nc.scalar.activation(att_vec[:], attn[:], AF.Tanh)
        trv2 = ttile([H, 128], db)
        nc.tensor.transpose(trv2[:], att_vec[:], identb[:])
        attvT_sb = wtile((H, 128), "attvT", db)
        nc.scalar.copy(attvT_sb[:], trv2[:])

        # ---- GATv2 x2 ----------------------------------------------------
        def gat_layer(XT_tile, prm, tag):
            p_xl = ptile([H, 128], 0)
            nc.tensor.matmul(p_xl[:], lhsT=prm["wlT"][:], rhs=XT_tile[:],
                             start=True, stop=True)
            xlT = wtile((H, 128), f"xlT{tag}", db)
            nc.scalar.activation(xlT[:], p_xl[:], AF.Identity, bias=prm["bl"][:])
            p_xr = ptile([H, 128], 1)
            nc.tensor.matmul(p_xr[:], lhsT=prm["wrT"][:], rhs=XT_tile[:],
                             start=True, stop=True)
            xrT = wtile((H, 128), f"xrT{tag}", db)
            nc.scalar.activation(xrT[:], p_xr[:], AF.Identity, bias=prm["br"][:])

            p_t = ttile([128, H], db)
            nc.tensor.transpose(p_t[:], xlT[:], identb[0:H, 0:H])
            xl_nat = wtile((128, H), f"xln{tag}", db)
            nc.scalar.copy(xl_nat[:], p_t[:])

            xlf = wtile((1, 128 * H), "xlf", db)
            xlf_ap = xlf[:]
            nc.gpsimd.dma_start(
                out=AP(tensor=xlf_ap.tensor, offset=xlf_ap.offset,
                       ap=[xlf_ap.ap[0], [H, 128], [1, H]]),
                in_=xl_nat[:],
            )
            e_sb = wtile((128, 128 * H), "e_sb")
            id_ap = identb[0:H, 0:H]
            id_rep = AP(tensor=id_ap.tensor, offset=id_ap.offset,
                        ap=[id_ap.ap[0], [0, 512 // H], id_ap.ap[1]])
            for j in range(4):
                p_e = ptile([128, 512], 2 + j)
                nc.tensor.matmul(p_e[:], lhsT=xrT[:], rhs=id_rep,
                                 start=True, stop=False)
                nc.tensor.matmul(p_e[:], lhsT=ones1[:],
                                 rhs=xlf[0:1, j * 512 : (j + 1) * 512],
                                 start=False, stop=True)
                # lrelu(v, 0.2) = 0.6*(v + (2/3)*|v|); the 0.6 is folded into
                # the host-side scaling of `a` (abc input carries 0.6*a).
                ab_t = wtile((128, 512), f"ab{j}")
                nc.scalar.activation(ab_t[:], p_e[:], AF.Abs)
                nc.vector.scalar_tensor_tensor(
                    out=e_sb[:, j * 512 : (j + 1) * 512], in0=ab_t[:],
                    scalar=2.0 / 3.0, in1=p_e[:], op0=OP.mult, op1=OP.add,
                )
            ew = wtile((128, 128 * H), "ew")
            abc_ap = prm["abc"][:]
            ew3 = ew[:].rearrange("p (s h) -> p s h", h=H)
            e3 = e_sb[:].rearrange("p (s h) -> p s h", h=H)
            # 2-input mul: DVE ~2x gpsimd -> 2/3 + 1/3 split
            nc.vector.tensor_mul(ew3[:, 0:84, :], e3[:, 0:84, :],
                                 bc_free(abc_ap, [[0, 84], [1, H]]))
            nc.gpsimd.tensor_mul(ew3[:, 84:128, :], e3[:, 84:128, :],
                                 bc_free(abc_ap, [[0, 44], [1, H]]))
            spre = wtile((128, 128), "spre")
            nc.vector.reduce_sum(out=spre[:, 0:84], in_=ew3[:, 0:84, :], axis=X)
            nc.vector.reduce_sum(out=spre[:, 84:128], in_=ew3[:, 84:128, :], axis=X)
            # |s_pre| is tiny: skip max-subtraction, fuse the sum into Exp
            ex2 = wtile((128, 128), "ex2")
            sm2 = wtile((128, 1), "sm2")
            nc.scalar.activation(ex2[:], spre[:], AF.Exp, accum_out=sm2[:])
            rs2 = wtile((128, 1), "rs2")
            nc.vector.reciprocal(rs2[:], sm2[:])
            alph = wtile((128, 128), "alph", db)
            nc.vector.tensor_scalar_mul(alph[:], ex2[:], rs2[:])
            p_at = ttile([128, 128], db)
            nc.tensor.transpose(p_at[:], alph[:], identb[:])
            alphT = wtile((128, 128), "alphT", db)
            nc.scalar.copy(alphT[:], p_at[:])
            p_g = ptile([H, 128], 0)
            nc.tensor.matmul(p_g[:], lhsT=xl_nat[:], rhs=alphT[:],
                             start=True, stop=True)
            gT = wtile((H, 128), f"gT{tag}", db)
            nc.scalar.activation(gT[:], p_g[:], AF.Relu, bias=prm["gb"][:])
            return gT

        g0T = gat_layer(attvT_sb, gat_sb[0], "0")
        g1T = gat_layer(g0T, gat_sb[1], "1")
        gsumT = wtile((H, 128), "gsumT", db)
        nc.vector.tensor_add(gsumT[:], g0T[:], g1T[:])

        # ---- capsule priors + routing ------------------------------------
        P1 = wtile((128, H * 128), "P1", db)  # [o, (l, c)] bf16
        for l in range(H):
            pc = ptile([128, 128], l % 2)
            nc.tensor.matmul(pc[:], lhsT=wc1T_sb[:, l * 128 : (l + 1) * 128],
                             rhs=attvT_sb[:], start=True, stop=False)
            nc.tensor.matmul(pc[:], lhsT=wc2T_sb[:, l * 128 : (l + 1) * 128],
                             rhs=gsumT[:], start=False, stop=True)
            nc.scalar.copy(P1[:, l * 128 : (l + 1) * 128], pc[:])

        P1_ap = P1[:]
        P1_lc = P1_ap.rearrange("p (l c) -> p l c", l=H)
        P1_cl = AP(tensor=P1_ap.tensor, offset=P1_ap.offset,
                   ap=[P1_ap.ap[0], [1, 128], [128, H]])

        def squash(v_tile, tag):
            sq = wtile((128, H), f"sq{tag}")
            n2 = wtile((128, 1), f"n2{tag}")
            nc.scalar.activation(sq[:], v_tile[:], AF.Square, accum_out=n2[:])
            st = wtile((128, 1), f"st{tag}")
            nc.scalar.activation(st[:], n2[:], AF.Sqrt, bias=eps_t[:])
            den = wtile((128, 1), f"den{tag}")
            nc.vector.scalar_tensor_tensor(out=den[:], in0=n2[:], scalar=1.0,
                                           in1=st[:], op0=OP.add, op1=OP.mult)
            rden = wtile((128, 1), f"rden{tag}")
            nc.vector.reciprocal(rden[:], den[:])
            coef = wtile((128, 1), f"coef{tag}")
            nc.vector.tensor_mul(coef[:], n2[:], rden[:])
            osq = wtile((128, H), f"osq{tag}")
            nc.vector.tensor_scalar_mul(osq[:], v_tile[:], coef[:])
            return osq

        def P1_cl_slice(c0, c1):
            return AP(tensor=P1_ap.tensor, offset=P1_ap.offset + c0,
                      ap=[P1_ap.ap[0], [1, c1 - c0], [128, H]])

        def delta_chain(osq, seed, tag):
            dw = wtile((128, 128 * H), "dw")
            dw3 = dw[:].rearrange("p (c l) -> p c l", l=H)
            nc.vector.tensor_mul(dw3[:, 0:84, :], P1_cl_slice(0, 84),
                                 bc_free(osq[:], [[0, 84], [1, H]]))
            nc.gpsimd.tensor_mul(dw3[:, 84:128, :], P1_cl_slice(84, 128),
                                 bc_free(osq[:], [[0, 44], [1, H]]))
            out_t = wtile((128, 128), f"dacc{tag}")
            nc.vector.reduce_sum(out=out_t[:, 0:84], in_=dw3[:, 0:84, :], axis=X)
            nc.vector.reduce_sum(out=out_t[:, 84:128], in_=dw3[:, 84:128, :], axis=X)
            if seed is not None:
                nc.vector.tensor_add(out_t[:], out_t[:], seed[:])
            return out_t

        # iter 0: p uniform = 1/128 -> out0 = squash(mean_c priors)
        o0 = wtile((128, H), "o0")
        nc.vector.reduce_sum(out=o0[:], in_=P1_lc, axis=X)
        o0b = wtile((128, H), "o0b")
        nc.vector.tensor_scalar_mul(o0b[:], o0[:], 1.0 / 128.0)
        o0 = o0b
        osq = squash(o0, "0")
        logits = delta_chain(osq, None, 0)

        for it in (1, 2):
            p_l = ttile([128, 128])
            nc.tensor.transpose(p_l[:], logits[:], ident[:])
            # logits are O(10): exp in fp32 cannot overflow -> no max pass
            exl = wtile((128, 128), "exl")
            sml = wtile((128, 1), "sml")
            nc.scalar.activation(exl[:], p_l[:], AF.Exp, accum_out=sml[:])
            rsl = wtile((128, 1), "rsl")
            nc.vector.reciprocal(rsl[:], sml[:])
            pco = wtile((128, 128), "pco", db)
            nc.vector.tensor_scalar_mul(pco[:], exl[:], rsl[:])
            p_p = ttile([128, 128], db)
            nc.tensor.transpose(p_p[:], pco[:], identb[:])
            pT = wtile((128, 128), "pT", db)
            nc.scalar.copy(pT[:], p_p[:])
            pw = wtile((128, 128 * H), "pw")
            pw3 = pw[:].rearrange("p (l c) -> p l c", l=H)
            nc.vector.tensor_mul(pw3, P1_lc, bc_free(pT[:], [[0, H], [1, 128]]))
            orr = wtile((128, H), "orr")
            nc.vector.reduce_sum(out=orr[:], in_=pw3, axis=X)
            osq = squash(orr, str(it))
            if it == 1:
                logits = delta_chain(osq, logits, 1)

        # ---- fusion output -----------------------------------------------
        ro = wtile((128, H), "ro", db)
        nc.scalar.activation(ro[:], osq[:], AF.Relu)
        p_ro = ttile([H, 128], db)
        nc.tensor.transpose(p_ro[:], ro[:], identb[:])
        roT = wtile((H, 128), "roT", db)
        nc.scalar.copy(roT[:], p_ro[:])
        p_f = ptile([128, 128], 2)
        nc.tensor.matmul(p_f[:], lhsT=roT[:], rhs=wfusT_sb[:], start=True, stop=True)
        fsum = wtile((128, 128), "fsum")
        nc.vector.tensor_add(fsum[:], p_f[:], bfus_sb[:])
        fout = wtile((128, 128), "fout")
        nc.scalar.activation(fout[:], fsum[:], AF.Tanh)
        nc.gpsimd.dma_start(out=out_d, in_=fout[:])

    nc.compile()
    return nc


# --------------------------------------------------------------------------
# host-side input prep / sharding
# --------------------------------------------------------------------------
def _colpack(vecs):
    """list of (128,) vectors -> (128, len) column array."""
    return np.stack([np.asarray(v, np.float32) for v in vecs], axis=1)


def _prep_in_maps(inputs):
    import ml_dtypes

    bf16 = ml_dtypes.bfloat16
    f32 = lambda k: np.asarray(inputs[k], np.float32)
    x = f32("inputs").reshape(T, F)
    xT_arr = np.ascontiguousarray(x.reshape(T, KT, 128).transpose(2, 1, 0)).reshape(
        128, KT * T
    )

    base = {"xT": xT_arr.astype(bf16)}
    base["wattT"] = np.ascontiguousarray(f32("W_att").T).astype(bf16)
    base["battbc"] = np.tile(f32("b_att").reshape(1, T), (128, 1))
    for layer, (wl, bl, wr, br, a, gb) in enumerate(
        [("Wl0", "bl0", "Wr0", "br0", "a0", "gb0"),
         ("Wl1", "bl1", "Wr1", "br1", "a1", "gb1")]
    ):
        base[f"wlT{layer}"] = np.ascontiguousarray(f32(wl).T).astype(bf16)
        base[f"wrT{layer}"] = np.ascontiguousarray(f32(wr).T).astype(bf16)
        base[f"bl{layer}"] = f32(bl).reshape(H, 1)
        base[f"br{layer}"] = f32(br).reshape(H, 1)
        base[f"gb{layer}"] = f32(gb).reshape(H, 1)
        base[f"abc{layer}"] = np.tile(0.6 * f32(a).reshape(1, H), (128, 1))
    wc = np.ascontiguousarray(f32("W_caps").transpose(2, 1, 0))  # (2H, H, I)
    base["wc1T"] = np.ascontiguousarray(wc[:H].reshape(H, H * I)).astype(bf16)
    base["wc2T"] = np.ascontiguousarray(wc[H:].reshape(H, H * I)).astype(bf16)
    base["wfusT"] = np.ascontiguousarray(f32("W_fus").T).astype(bf16)
    base["bfus"] = np.tile(f32("b_fus").reshape(1, I), (I, 1))

    in_maps = []
    for c in range(NCORES):
        m = dict(base)
        # layer 0: contiguous feature slice; layer 1: h-major permuted order
        # (slot u*128+p holds feature p*16 + 2c+u) so the gathered emb is
        # h-major and the attention output lands directly in (i, h) layout.
        idx_by_layer = [
            np.arange(c * GPC, (c + 1) * GPC),
            np.array([p * H + 2 * c + u for u in range(2) for p in range(128)]),
        ]
        for layer, (wk, bik, bhk) in enumerate(
            [("W_ih0", "b_ih0", "b_hh0"), ("W_ih1", "b_ih1", "b_hh1")]
        ):
            idx = idx_by_layer[layer]
            W = f32(wk)
            Wc = np.concatenate([W[g * F + idx] for g in range(3)], axis=0)
            wfull = np.ascontiguousarray(
                Wc.reshape(3 * GPC, KT, 128).transpose(2, 1, 0)
            ).reshape(128, KT * 3 * GPC).astype(bf16)
            step = wfull.shape[1] // NCHUNK
            for j in range(NCHUNK):
                m[f"w{layer}_{j}"] = np.ascontiguousarray(
                    wfull[:, j * step : (j + 1) * step]
                )
            bih, bhh = f32(bik), f32(bhk)
            cols = []
            for b in range(2):
                ib = idx[b * 128 : (b + 1) * 128]
                cols.append(bih[0 * F + ib] + bhh[0 * F + ib])
            for b in range(2):
                ib = idx[b * 128 : (b + 1) * 128]
                cols.append(-(bih[1 * F + ib] + bhh[1 * F + ib]))
            for b in range(2):
                ib = idx[b * 128 : (b + 1) * 128]
                cols.append(bih[2 * F + ib])
            for b in range(2):
                ib = idx[b * 128 : (b + 1) * 128]
                cols.append(bhh[2 * F + ib])
            m[f"bias{layer}"] = _colpack(cols)
        in_maps.append(m)
    return in_maps


def kernel(**inputs):
    if "nc" not in _STATE:
        _STATE["nc"] = _build_module()
    nc = _STATE["nc"]
    in_maps = _prep_in_maps(inputs)

    from concourse.bass_utils import run_bass_kernel_spmd

    trace = bool(int(os.environ.get("KERNEL_TRACE", "0")))
    res = run_bass_kernel_spmd(nc, in_maps, core_ids=list(range(NCORES)), trace=trace)
    _STATE["last_results"] = res
    return np.asarray(res.results[0]["out"], np.float32).reshape(1, I, I)
